# revision 1
# baseline (speedup 1.0000x reference)
"""GPTQ 4-bit quantized linear on 8 Trainium2 NeuronCores.

y[b,s,o] = sum_i x[b,s,i] * W[o,i] + bias[o]
  W[o,i] = (nib(qweight)[o,i] - zeros[o,i//128]) * scales[o,i//128]
  qweight int32 packs 2 nibbles in its low byte: i=2j low, i=2j+1 high.

Sharding: 4-way over out_features x 2-way over tokens (8 cores).
Per core: out shard [4096 tokens, 1024 outs].

Device kernel per core:
  - W dequant in natural [o_part, i_free] layout (scale/zero are
    per-partition there), then PE-transpose 128x128 blocks into
    WT k-tiles [i_part, o_free] resident in SBUF (16.8 MB).
  - x arrives transposed [in_f, tokens] (host-side layout prep);
    streamed as [128, chunk] k-tiles.
  - Matmul out[t,o] = sum_k xT_k.T @ WT_k accumulating in PSUM,
    float32r (1 cycle/row at N>=256), bias added on PSUM->SBUF copy.
"""

from contextlib import ExitStack

import numpy as np

import concourse.bass as bass
import concourse.mybir as mybir
import concourse.tile as tile
from concourse.bass_utils import run_bass_kernel_spmd
from concourse.masks import make_identity

F32 = mybir.dt.float32
F32R = mybir.dt.float32r
I32 = mybir.dt.int32
AF = mybir.ActivationFunctionType
ALU = mybir.AluOpType

# Problem shape (hardcoded; kernel.py must be self-contained).
B, S, IN, OUT = 4, 2048, 4096, 4096
TOK = B * S
GROUP = 128
O_WAYS, T_WAYS = 4, 2
N_CORES = 8


def build_nc(
    tsh=TOK // T_WAYS,   # tokens per core
    in_f=IN,             # contraction size
    osh=OUT // O_WAYS,   # out features per core
    chunk=256,           # tokens per pipeline chunk
):
    assert in_f % 256 == 0 and osh % 128 == 0 and tsh % chunk == 0
    assert chunk % 128 == 0
    nk = in_f // 128           # k tiles (also = number of quant groups)
    n_osub = osh // 128        # 128-row o blocks for dequant
    rhs_w = min(512, osh)      # matmul moving width
    n_rhs = osh // rhs_w
    n_tsub = chunk // 128
    n_chunk = tsh // chunk
    half = in_f // 2           # packed j count
    qq_j = min(512, half)      # j columns per dequant block
    n_qq = half // qq_j
    g_per_qq = (2 * qq_j) // GROUP
    ng = in_f // GROUP

    nc = bass.Bass()
    xt_d = nc.declare_dram_parameter("xt", [in_f, tsh], F32R, isOutput=False)
    qw = nc.declare_dram_parameter("qw", [osh, half], I32, isOutput=False)
    sc = nc.declare_dram_parameter("sc", [osh, ng], F32, isOutput=False)
    nz = nc.declare_dram_parameter("nz", [osh, ng], F32, isOutput=False)
    bi = nc.declare_dram_parameter("bi", [128, osh], F32, isOutput=False)
    out = nc.declare_dram_parameter("out", [tsh, osh], F32, isOutput=True)

    with tile.TileContext(nc) as tc, ExitStack() as ctx:
        P = 128
        pool_const = ctx.enter_context(tc.tile_pool(name="const", bufs=1))
        pool_wt = ctx.enter_context(tc.tile_pool(name="wt", bufs=1))
        pool_q = ctx.enter_context(tc.tile_pool(name="q", bufs=2))
        pool_wi = ctx.enter_context(tc.tile_pool(name="wi", bufs=2))
        pool_wf = ctx.enter_context(tc.tile_pool(name="wf", bufs=2))
        pool_ss = ctx.enter_context(tc.tile_pool(name="ss", bufs=2))
        pool_x = ctx.enter_context(tc.tile_pool(name="x", bufs=6))
        pool_ob = ctx.enter_context(tc.tile_pool(name="ob", bufs=4))
        psum_w = ctx.enter_context(tc.tile_pool(name="psw", bufs=2, space="PSUM"))
        psum_mm = ctx.enter_context(tc.tile_pool(name="psm", bufs=6, space="PSUM"))

        ident = pool_const.tile([P, P], F32, tag="ident")
        make_identity(nc, ident[:])

        bias_t = pool_const.tile([P, osh], F32, tag="bias")
        nc.gpsimd.dma_start(out=bias_t[:], in_=bi[:, :])

        # Persistent dequantized W^T k-tiles.
        WT = [
            pool_wt.tile([P, osh], F32R, tag=f"wt{k}", name=f"wt{k}")
            for k in range(nk)
        ]

        # ---- W build: unpack + dequant (natural layout) + PE transpose ----
        for qq in range(n_qq):
            for osub in range(n_osub):
                op = osub * P
                s_t = pool_ss.tile([P, ng], F32, tag="s")
                nz_t = pool_ss.tile([P, ng], F32, tag="nz")
                nc.gpsimd.dma_start(out=s_t[:], in_=sc[op : op + P, :])
                nc.gpsimd.dma_start(out=nz_t[:], in_=nz[op : op + P, :])

                q_t = pool_q.tile([P, qq_j], I32, tag="q")
                nc.gpsimd.dma_start(
                    out=q_t[:], in_=qw[op : op + P, qq * qq_j : (qq + 1) * qq_j]
                )
                wi_t = pool_wi.tile([P, 2 * qq_j], I32, tag="wi")
                wi3 = wi_t[:].rearrange("p (j a) -> p a j", a=2)
                # even i: low nibble; odd i: high nibble
                nc.vector.tensor_scalar(
                    wi3[:, 0, :], q_t[:], 15, None, ALU.bitwise_and
                )
                nc.vector.tensor_scalar(
                    wi3[:, 1, :], q_t[:], 4, 15,
                    ALU.logical_shift_right, ALU.bitwise_and,
                )
                wf_t = pool_wf.tile([P, 2 * qq_j], F32, tag="wf")
                nc.vector.tensor_copy(wf_t[:], wi_t[:])
                for g in range(g_per_qq):
                    gg = qq * g_per_qq + g
                    nc.vector.tensor_mul(
                        wf_t[:, g * GROUP : (g + 1) * GROUP],
                        wf_t[:, g * GROUP : (g + 1) * GROUP],
                        s_t[:, gg : gg + 1].to_broadcast([128, GROUP]),
                    )
                    nc.vector.tensor_add(
                        wf_t[:, g * GROUP : (g + 1) * GROUP],
                        wf_t[:, g * GROUP : (g + 1) * GROUP],
                        nz_t[:, gg : gg + 1].to_broadcast([128, GROUP]),
                    )
                # transpose each 128x128 block into its WT k-tile column
                for g in range(g_per_qq):
                    k = qq * g_per_qq + g
                    pw = psum_w.tile([P, P], F32, tag="pw", name=f"pw{qq}_{osub}_{g}")
                    nc.tensor.transpose(
                        pw[:], wf_t[:, g * GROUP : (g + 1) * GROUP], ident[:]
                    )
                    nc.vector.tensor_copy(WT[k][:, op : op + P], pw[:])

        # ---- main loop: stream x^T chunks, matmul, bias, store ----
        for ch in range(n_chunk):
            t0 = ch * chunk
            xts = []
            for k in range(nk):
                xt = pool_x.tile([P, chunk], F32R, tag="xt", name=f"xt{ch}_{k}")
                nc.sync.dma_start(
                    out=xt[:], in_=xt_d[k * P : (k + 1) * P, t0 : t0 + chunk]
                )
                xts.append(xt)
            ps = [
                [
                    psum_mm.tile([P, rhs_w], F32, tag="ps", name=f"ps{ch}_{t}_{r}")
                    for r in range(n_rhs)
                ]
                for t in range(n_tsub)
            ]
            for k in range(nk):
                for tsub in range(n_tsub):
                    lhsT = xts[k][:, tsub * P : (tsub + 1) * P]
                    for r in range(n_rhs):
                        nc.tensor.matmul(
                            ps[tsub][r][:],
                            lhsT,
                            WT[k][:, r * rhs_w : (r + 1) * rhs_w],
                            start=(k == 0),
                            stop=(k == nk - 1),
                        )
            for tsub in range(n_tsub):
                ob = pool_ob.tile([P, osh], F32, tag="ob", name=f"ob{ch}_{tsub}")
                for r in range(n_rhs):
                    nc.vector.tensor_add(
                        ob[:, r * rhs_w : (r + 1) * rhs_w],
                        ps[tsub][r][:],
                        bias_t[:, r * rhs_w : (r + 1) * rhs_w],
                    )
                nc.scalar.dma_start(
                    out=out[t0 + tsub * P : t0 + (tsub + 1) * P, :], in_=ob[:]
                )
    _legalize_waits(nc)
    return nc


_SPLIT_TYPES = (
    "InstTensorTensor",
    "InstTensorScalarPtr",
    "InstTensorScalar",
    "InstActivation",
    "InstTensorCopy",
    "InstMatmult",
    "InstDMACopy",
    "InstDrain",
)


def _legalize_waits(nc):
    """walrus allows only one on-inst sync wait for DVE/ACT elementwise
    instruction encodings; split extra waits onto same-engine Drains."""
    f = nc.m.functions[0]
    n = 0
    for blk in f.blocks:
        out_insts = []
        for inst in blk.instructions:
            si = inst.sync_info
            if (
                si is not None
                and len(si.on_wait) > 1
                and type(inst).__name__ in _SPLIT_TYPES
            ):
                waits = list(si.on_wait)
                for w in waits[:-1]:
                    d = mybir.InstDrain(name=f"waitfix{n}", ins=[], outs=[])
                    d.engine = inst.engine
                    d.sync_info = mybir.SyncInfo(on_wait=[w], on_update=[])
                    out_insts.append(d)
                    n += 1
                inst.sync_info = mybir.SyncInfo(
                    on_wait=[waits[-1]], on_update=list(si.on_update)
                )
            out_insts.append(inst)
        blk.instructions = out_insts


_NC_CACHE = {}


def _get_nc(key=()):
    if key not in _NC_CACHE:
        _NC_CACHE[key] = build_nc(*key) if key else build_nc()
    return _NC_CACHE[key]


def make_in_maps(x, qweight, scales, zeros, bias):
    x2 = x.reshape(TOK, IN)
    tsh = TOK // T_WAYS
    osh = OUT // O_WAYS
    # Host-side layout prep: transpose each token shard once; shared by
    # the 4 cores that consume it.
    xt_shards = [
        np.ascontiguousarray(x2[t * tsh : (t + 1) * tsh].T) for t in range(T_WAYS)
    ]
    in_maps = []
    for c in range(N_CORES):
        o0 = (c % O_WAYS) * osh
        sc_s = np.ascontiguousarray(scales[o0 : o0 + osh])
        in_maps.append(
            {
                "xt": xt_shards[c // O_WAYS],
                "qw": np.ascontiguousarray(qweight[o0 : o0 + osh]),
                "sc": sc_s,
                "nz": -(zeros[o0 : o0 + osh].astype(np.float32) * sc_s),
                "bi": np.ascontiguousarray(
                    np.broadcast_to(bias[o0 : o0 + osh], (128, osh))
                ),
            }
        )
    return in_maps


def _run(x, qweight, scales, zeros, bias, trace=False, **kw):
    nc = _get_nc()
    in_maps = make_in_maps(x, qweight, scales, zeros, bias)
    res = run_bass_kernel_spmd(nc, in_maps, list(range(N_CORES)), trace=trace, **kw)
    tsh = TOK // T_WAYS
    osh = OUT // O_WAYS
    full = np.empty((TOK, OUT), dtype=np.float32)
    for c in range(N_CORES):
        o0 = (c % O_WAYS) * osh
        t0 = (c // O_WAYS) * tsh
        full[t0 : t0 + tsh, o0 : o0 + osh] = res.results[c]["out"]
    return full.reshape(B, S, OUT), res


def kernel(x, qweight, scales, zeros, bias):
    out, _ = _run(x, qweight, scales, zeros, bias)
    return out



# revision 2
# speedup vs baseline: 1.8117x; 1.8117x over previous
"""GPTQ 4-bit quantized linear on 8 Trainium2 NeuronCores.

y[b,s,o] = sum_i x[b,s,i] * W[o,i] + bias[o]
  W[o,i] = (nib(qweight)[o,i] - zeros[o,i//128]) * scales[o,i//128]
  qweight int32 packs 2 nibbles in its low byte: i=2j low, i=2j+1 high.

Sharding: 4-way over out_features x 2-way over tokens (8 cores).
Per core: out shard [4096 tokens, 1024 outs].

Strategy (v2):
  - All layout work on host: nibbles pre-unpacked to u8 in W^T [i, o]
    order, scales/-z*s pre-broadcast to [128, osh] rows, x transposed,
    bf16-cast and chunk-major so every DMA is one fat contiguous run
    per partition.
  - The contraction index i is permuted (same permutation on x rows and
    W rows, so the matmul is unchanged) such that two k-slots share one
    scale-broadcast tile: k-slot pair (2m, 2m+1) rows 0-63 are from
    quant group 2m and rows 64-127 from group 2m+1.
  - Device W build per k-slot: WT[k] = qn_u8 * s_bc + nzs_bc, two DVE
    passes, bf16 out. No PE transposes, no PSUM use outside matmuls.
  - Main loop: 8 chunks of 512 tokens; per chunk one big x DMA
    (32KB/partition contiguous), then tsub-outer / k-inner matmuls
    (bf16, N=512) accumulating in 8 PSUM banks; bias added on the
    PSUM->SBUF drain (DVE); stores on the scalar HWDGE queue.
"""

from contextlib import ExitStack

import numpy as np
import ml_dtypes

import concourse.bass as bass
import concourse.mybir as mybir
import concourse.tile as tile
from concourse.bass_utils import run_bass_kernel_spmd

F32 = mybir.dt.float32
BF16 = mybir.dt.bfloat16
U8 = mybir.dt.uint8

# Problem shape (hardcoded; kernel.py must be self-contained).
B, S, IN, OUT = 4, 2048, 4096, 4096
TOK = B * S
GROUP = 128
O_WAYS, T_WAYS = 4, 2
N_CORES = 8

TSH = TOK // T_WAYS      # tokens per core (4096)
OSH = OUT // O_WAYS      # out features per core (1024)
NK = IN // 128           # k slots (32)
NPAIR = NK // 2          # 16
CHUNK = 512              # tokens per chunk
N_CHUNK = TSH // CHUNK   # 8
N_TSUB = CHUNK // 128    # 4
RHS_W = 512
N_RHS = OSH // RHS_W     # 2

BF = ml_dtypes.bfloat16


def _perm():
    """i-permutation: device row r = 128*kslot + p maps to original i.

    k-slot pair (2m, 2m+1): rows 0-63 from group 2m, rows 64-127 from
    group 2m+1, so one [128, OSH] scale tile serves both k-slots.
    """
    perm = np.empty(IN, np.int64)
    p = np.arange(64)
    for m in range(NPAIR):
        base = 256 * m
        perm[128 * (2 * m) + p] = base + p
        perm[128 * (2 * m) + 64 + p] = base + 128 + p
        perm[128 * (2 * m + 1) + p] = base + 64 + p
        perm[128 * (2 * m + 1) + 64 + p] = base + 192 + p
    return perm


PERM = _perm()


def build_nc():
    nc = bass.Bass()
    xt_d = nc.declare_dram_parameter("xt", [N_CHUNK, 128, NK * CHUNK], BF16, isOutput=False)
    qn_d = nc.declare_dram_parameter("qn", [NPAIR, 128, 2 * OSH], U8, isOutput=False)
    szb_d = nc.declare_dram_parameter("szb", [NPAIR, 128, 2 * OSH], BF16, isOutput=False)
    bi_d = nc.declare_dram_parameter("bi", [128, OSH], F32, isOutput=False)
    out_d = nc.declare_dram_parameter("out", [TSH, OSH], F32, isOutput=True)

    with tile.TileContext(nc) as tc, ExitStack() as ctx:
        P = 128
        pool_const = ctx.enter_context(tc.tile_pool(name="const", bufs=1))
        pool_wt = ctx.enter_context(tc.tile_pool(name="wt", bufs=1))
        pool_qn = ctx.enter_context(tc.tile_pool(name="qn", bufs=3))
        pool_sz = ctx.enter_context(tc.tile_pool(name="sz", bufs=3))
        pool_x = ctx.enter_context(tc.tile_pool(name="x", bufs=2))
        pool_ob = ctx.enter_context(tc.tile_pool(name="ob", bufs=4))
        psum_mm = ctx.enter_context(tc.tile_pool(name="psm", bufs=8, space="PSUM"))

        bias_t = pool_const.tile([P, OSH], F32, tag="bias")
        nc.gpsimd.dma_start(out=bias_t[:], in_=bi_d[:, :])

        # ---- W build: WT[k] = qn * s_bc + nzs_bc (all bf16, DVE) ----
        WT = [None] * NK
        for m in range(NPAIR):
            qn2 = pool_qn.tile([P, 2 * OSH], U8, tag="qn", name=f"qn{m}")
            nc.gpsimd.dma_start(out=qn2[:], in_=qn_d[m, :, :])
            sz = pool_sz.tile([P, 2 * OSH], BF16, tag="sz", name=f"sz{m}")
            nc.scalar.dma_start(out=sz[:], in_=szb_d[m, :, :])
            for t in range(2):
                k = 2 * m + t
                wt = pool_wt.tile([P, OSH], BF16, tag=f"wt{k}", name=f"wt{k}")
                nc.vector.tensor_mul(
                    wt[:], qn2[:, t * OSH : (t + 1) * OSH], sz[:, 0:OSH]
                )
                nc.vector.tensor_add(wt[:], wt[:], sz[:, OSH : 2 * OSH])
                WT[k] = wt

        # ---- main loop: 8 chunks of 512 tokens ----
        for ch in range(N_CHUNK):
            xc = pool_x.tile([P, NK * CHUNK], BF16, tag="xc", name=f"xc{ch}")
            nsplit = 4 if ch == 0 else 2
            step = (NK * CHUNK) // nsplit
            for s in range(nsplit):
                nc.sync.dma_start(
                    out=xc[:, s * step : (s + 1) * step],
                    in_=xt_d[ch, :, s * step : (s + 1) * step],
                )
            for tsub in range(N_TSUB):
                ps = [
                    psum_mm.tile([P, RHS_W], F32, tag="ps", name=f"ps{ch}_{tsub}_{r}")
                    for r in range(N_RHS)
                ]
                for k in range(NK):
                    lhsT = xc[:, k * CHUNK + tsub * P : k * CHUNK + (tsub + 1) * P]
                    for r in range(N_RHS):
                        nc.tensor.matmul(
                            ps[r][:],
                            lhsT,
                            WT[k][:, r * RHS_W : (r + 1) * RHS_W],
                            start=(k == 0),
                            stop=(k == NK - 1),
                        )
                ob = pool_ob.tile([P, OSH], F32, tag="ob", name=f"ob{ch}_{tsub}")
                for r in range(N_RHS):
                    nc.vector.tensor_add(
                        ob[:, r * RHS_W : (r + 1) * RHS_W],
                        ps[r][:],
                        bias_t[:, r * RHS_W : (r + 1) * RHS_W],
                    )
                t0 = ch * CHUNK + tsub * P
                nc.scalar.dma_start(out=out_d[t0 : t0 + P, :], in_=ob[:])
    _legalize_waits(nc)
    return nc


_SPLIT_TYPES = (
    "InstTensorTensor",
    "InstTensorScalarPtr",
    "InstTensorScalar",
    "InstActivation",
    "InstTensorCopy",
    "InstMatmult",
    "InstDMACopy",
    "InstDrain",
)


def _legalize_waits(nc):
    """walrus allows only one on-inst sync wait for DVE/ACT elementwise
    instruction encodings; split extra waits onto same-engine Drains."""
    f = nc.m.functions[0]
    n = 0
    for blk in f.blocks:
        out_insts = []
        for inst in blk.instructions:
            si = inst.sync_info
            if (
                si is not None
                and len(si.on_wait) > 1
                and type(inst).__name__ in _SPLIT_TYPES
            ):
                waits = list(si.on_wait)
                for w in waits[:-1]:
                    d = mybir.InstDrain(name=f"waitfix{n}", ins=[], outs=[])
                    d.engine = inst.engine
                    d.sync_info = mybir.SyncInfo(on_wait=[w], on_update=[])
                    out_insts.append(d)
                    n += 1
                inst.sync_info = mybir.SyncInfo(
                    on_wait=[waits[-1]], on_update=list(si.on_update)
                )
            out_insts.append(inst)
        blk.instructions = out_insts


_NC_CACHE = {}


def _get_nc(key=()):
    if key not in _NC_CACHE:
        _NC_CACHE[key] = build_nc()
    return _NC_CACHE[key]


def make_in_maps(x, qweight, scales, zeros, bias):
    x2 = np.asarray(x).reshape(TOK, IN)
    qweight = np.asarray(qweight)
    scales = np.asarray(scales)
    zeros = np.asarray(zeros)
    bias = np.asarray(bias)

    # x: per token-shard, transpose, permute rows, bf16, chunk-major:
    # [ch, p, k, t] so each partition's per-chunk data is contiguous.
    xt_shards = []
    for t in range(T_WAYS):
        xs = x2[t * TSH : (t + 1) * TSH]          # [TSH, IN]
        xtp = np.ascontiguousarray(xs.T[PERM]).astype(BF)   # [IN, TSH]
        xtp = xtp.reshape(NK, 128, N_CHUNK, CHUNK).transpose(2, 1, 0, 3)
        xt_shards.append(np.ascontiguousarray(xtp.reshape(N_CHUNK, 128, NK * CHUNK)))

    in_maps = []
    for c in range(N_CORES):
        o0 = (c % O_WAYS) * OSH
        qw = qweight[o0 : o0 + OSH]               # [OSH, IN//2] int32
        nib = np.empty((OSH, IN), np.uint8)
        nib[:, 0::2] = (qw & 15).astype(np.uint8)
        nib[:, 1::2] = ((qw >> 4) & 15).astype(np.uint8)
        qnT = nib.T[PERM]                          # [IN, OSH] u8
        qn_d = np.ascontiguousarray(
            qnT.reshape(NPAIR, 2, 128, OSH).transpose(0, 2, 1, 3).reshape(NPAIR, 128, 2 * OSH)
        )

        s = scales[o0 : o0 + OSH].astype(np.float32)       # [OSH, 32]
        nzs = -(zeros[o0 : o0 + OSH].astype(np.float32) * s)
        szb = np.empty((NPAIR, 128, 2 * OSH), np.float32)
        for m in range(NPAIR):
            szb[m, :64, 0:OSH] = s[:, 2 * m]
            szb[m, 64:, 0:OSH] = s[:, 2 * m + 1]
            szb[m, :64, OSH:] = nzs[:, 2 * m]
            szb[m, 64:, OSH:] = nzs[:, 2 * m + 1]

        in_maps.append(
            {
                "xt": xt_shards[c // O_WAYS],
                "qn": qn_d,
                "szb": szb.astype(BF),
                "bi": np.ascontiguousarray(
                    np.broadcast_to(bias[o0 : o0 + OSH], (128, OSH))
                ).astype(np.float32),
            }
        )
    return in_maps


def _run(x, qweight, scales, zeros, bias, trace=False, **kw):
    nc = _get_nc()
    in_maps = make_in_maps(x, qweight, scales, zeros, bias)
    res = run_bass_kernel_spmd(nc, in_maps, list(range(N_CORES)), trace=trace, **kw)
    full = np.empty((TOK, OUT), dtype=np.float32)
    for c in range(N_CORES):
        o0 = (c % O_WAYS) * OSH
        t0 = (c // O_WAYS) * TSH
        full[t0 : t0 + TSH, o0 : o0 + OSH] = res.results[c]["out"]
    return full.reshape(B, S, OUT), res


def kernel(x, qweight, scales, zeros, bias):
    out, _ = _run(x, qweight, scales, zeros, bias)
    return out


# revision 3
# speedup vs baseline: 1.8181x; 1.0036x over previous
"""GPTQ 4-bit quantized linear on 8 Trainium2 NeuronCores.

y[b,s,o] = sum_i x[b,s,i] * W[o,i] + bias[o]
  W[o,i] = (nib(qweight)[o,i] - zeros[o,i//128]) * scales[o,i//128]
  qweight int32 packs 2 nibbles in its low byte: i=2j low, i=2j+1 high.

Sharding: 4-way over out_features x 2-way over tokens (8 cores).
Per core: out shard [4096 tokens, 1024 outs].

Strategy (v3):
  - Weight prepacking on host: dequantize to bf16 W^T [i, o] tiles and
    lay x out transposed, bf16, chunk-major — every DMA is one fat
    contiguous run per partition (32KB/partition for x chunks).
  - Device: stream W^T k-tiles (resident, 64KB/partition total) on two
    queues, stream x in 8 chunks of 512 tokens on the sync queue, and
    run tsub-outer / k-inner bf16 matmuls (N=512) accumulating into all
    8 PSUM banks. Bias is added on the PSUM->SBUF drain (DVE); stores
    issue per 512-column half to start the final writeback earlier.
"""

from contextlib import ExitStack

import numpy as np
import ml_dtypes

import concourse.bass as bass
import concourse.mybir as mybir
import concourse.tile as tile
from concourse.bass_utils import run_bass_kernel_spmd

F32 = mybir.dt.float32
BF16 = mybir.dt.bfloat16

# Problem shape (hardcoded; kernel.py must be self-contained).
B, S, IN, OUT = 4, 2048, 4096, 4096
TOK = B * S
GROUP = 128
O_WAYS, T_WAYS = 4, 2
N_CORES = 8

TSH = TOK // T_WAYS      # tokens per core (4096)
OSH = OUT // O_WAYS      # out features per core (1024)
NK = IN // 128           # k slots (32)
NPAIR = NK // 2          # 16 (W streams in pairs of k slots)
CHUNK = 512              # tokens per chunk
N_CHUNK = TSH // CHUNK   # 8
N_TSUB = CHUNK // 128    # 4
RHS_W = 512
N_RHS = OSH // RHS_W     # 2

BF = ml_dtypes.bfloat16


def build_nc():
    nc = bass.Bass()
    xt_d = nc.declare_dram_parameter("xt", [N_CHUNK, 128, NK * CHUNK], BF16, isOutput=False)
    wt_d = nc.declare_dram_parameter("wt", [NPAIR, 128, 2 * OSH], BF16, isOutput=False)
    bi_d = nc.declare_dram_parameter("bi", [128, OSH], F32, isOutput=False)
    out_d = nc.declare_dram_parameter("out", [TSH, OSH], F32, isOutput=True)

    with tile.TileContext(nc) as tc, ExitStack() as ctx:
        P = 128
        pool_const = ctx.enter_context(tc.tile_pool(name="const", bufs=1))
        pool_wt = ctx.enter_context(tc.tile_pool(name="wt", bufs=1))
        pool_x = ctx.enter_context(tc.tile_pool(name="x", bufs=2))
        pool_ob = ctx.enter_context(tc.tile_pool(name="ob", bufs=4))
        psum_mm = ctx.enter_context(tc.tile_pool(name="psm", bufs=8, space="PSUM"))

        bias_t = pool_const.tile([P, OSH], F32, tag="bias")
        nc.gpsimd.dma_start(out=bias_t[:], in_=bi_d[:, :])

        # ---- W load: bf16 W^T pair-tiles, alternating scalar/gpsimd ----
        WTP = []
        for m in range(NPAIR):
            wtp = pool_wt.tile([P, 2 * OSH], BF16, tag=f"wt{m}", name=f"wt{m}")
            eng = nc.scalar if m % 2 == 0 else nc.gpsimd
            eng.dma_start(out=wtp[:], in_=wt_d[m, :, :])
            WTP.append(wtp)

        def WT(k):
            m, t = divmod(k, 2)
            return WTP[m][:, t * OSH : (t + 1) * OSH]

        # ---- main loop: 8 chunks of 512 tokens ----
        for ch in range(N_CHUNK):
            xc = pool_x.tile([P, NK * CHUNK], BF16, tag="xc", name=f"xc{ch}")
            nsplit = 4 if ch == 0 else 2
            step = (NK * CHUNK) // nsplit
            for s in range(nsplit):
                nc.sync.dma_start(
                    out=xc[:, s * step : (s + 1) * step],
                    in_=xt_d[ch, :, s * step : (s + 1) * step],
                )
            for tsub in range(N_TSUB):
                ps = [
                    psum_mm.tile([P, RHS_W], F32, tag="ps", name=f"ps{ch}_{tsub}_{r}")
                    for r in range(N_RHS)
                ]
                for k in range(NK):
                    lhsT = xc[:, k * CHUNK + tsub * P : k * CHUNK + (tsub + 1) * P]
                    for r in range(N_RHS):
                        nc.tensor.matmul(
                            ps[r][:],
                            lhsT,
                            WT(k)[:, r * RHS_W : (r + 1) * RHS_W],
                            start=(k == 0),
                            stop=(k == NK - 1),
                        )
                ob = pool_ob.tile([P, OSH], F32, tag="ob", name=f"ob{ch}_{tsub}")
                t0 = ch * CHUNK + tsub * P
                for r in range(N_RHS):
                    nc.vector.tensor_add(
                        ob[:, r * RHS_W : (r + 1) * RHS_W],
                        ps[r][:],
                        bias_t[:, r * RHS_W : (r + 1) * RHS_W],
                    )
                    nc.scalar.dma_start(
                        out=out_d[t0 : t0 + P, r * RHS_W : (r + 1) * RHS_W],
                        in_=ob[:, r * RHS_W : (r + 1) * RHS_W],
                    )
    _legalize_waits(nc)
    return nc


_SPLIT_TYPES = (
    "InstTensorTensor",
    "InstTensorScalarPtr",
    "InstTensorScalar",
    "InstActivation",
    "InstTensorCopy",
    "InstMatmult",
    "InstDMACopy",
    "InstDrain",
)


def _legalize_waits(nc):
    """walrus allows only one on-inst sync wait for DVE/ACT elementwise
    instruction encodings; split extra waits onto same-engine Drains."""
    f = nc.m.functions[0]
    n = 0
    for blk in f.blocks:
        out_insts = []
        for inst in blk.instructions:
            si = inst.sync_info
            if (
                si is not None
                and len(si.on_wait) > 1
                and type(inst).__name__ in _SPLIT_TYPES
            ):
                waits = list(si.on_wait)
                for w in waits[:-1]:
                    d = mybir.InstDrain(name=f"waitfix{n}", ins=[], outs=[])
                    d.engine = inst.engine
                    d.sync_info = mybir.SyncInfo(on_wait=[w], on_update=[])
                    out_insts.append(d)
                    n += 1
                inst.sync_info = mybir.SyncInfo(
                    on_wait=[waits[-1]], on_update=list(si.on_update)
                )
            out_insts.append(inst)
        blk.instructions = out_insts


_NC_CACHE = {}


def _get_nc(key=()):
    if key not in _NC_CACHE:
        _NC_CACHE[key] = build_nc()
    return _NC_CACHE[key]


def make_in_maps(x, qweight, scales, zeros, bias):
    x2 = np.asarray(x).reshape(TOK, IN)
    qweight = np.asarray(qweight)
    scales = np.asarray(scales)
    zeros = np.asarray(zeros)
    bias = np.asarray(bias)

    # x: per token-shard, transpose, bf16, chunk-major [ch, p, k, t] so
    # each partition's per-chunk data is one contiguous 32KB run.
    xt_shards = []
    for t in range(T_WAYS):
        xs = x2[t * TSH : (t + 1) * TSH]                     # [TSH, IN]
        xtp = np.ascontiguousarray(xs.T).astype(BF)          # [IN, TSH]
        xtp = xtp.reshape(NK, 128, N_CHUNK, CHUNK).transpose(2, 1, 0, 3)
        xt_shards.append(np.ascontiguousarray(xtp.reshape(N_CHUNK, 128, NK * CHUNK)))

    in_maps = []
    for c in range(N_CORES):
        o0 = (c % O_WAYS) * OSH
        qw = qweight[o0 : o0 + OSH]                          # [OSH, IN//2] int32
        nib = np.empty((OSH, IN), np.float32)
        nib[:, 0::2] = (qw & 15).astype(np.float32)
        nib[:, 1::2] = ((qw >> 4) & 15).astype(np.float32)
        s = scales[o0 : o0 + OSH].astype(np.float32)         # [OSH, 32]
        z = zeros[o0 : o0 + OSH].astype(np.float32)
        w = (nib.reshape(OSH, NK, GROUP) - z[:, :, None]) * s[:, :, None]
        wt = w.reshape(OSH, IN).T.astype(BF)                 # [IN, OSH] bf16
        wt_p = np.ascontiguousarray(wt.reshape(NPAIR, 2 * 128 * OSH)).reshape(
            NPAIR, 2, 128, OSH
        )
        # pair tile layout: [m, p, (t, o)] — k-slot 2m in cols 0:OSH,
        # k-slot 2m+1 in cols OSH:2*OSH
        wt_d = np.ascontiguousarray(
            wt_p.transpose(0, 2, 1, 3).reshape(NPAIR, 128, 2 * OSH)
        )

        in_maps.append(
            {
                "xt": xt_shards[c // O_WAYS],
                "wt": wt_d,
                "bi": np.ascontiguousarray(
                    np.broadcast_to(bias[o0 : o0 + OSH], (128, OSH))
                ).astype(np.float32),
            }
        )
    return in_maps


def _run(x, qweight, scales, zeros, bias, trace=False, **kw):
    nc = _get_nc()
    in_maps = make_in_maps(x, qweight, scales, zeros, bias)
    res = run_bass_kernel_spmd(nc, in_maps, list(range(N_CORES)), trace=trace, **kw)
    full = np.empty((TOK, OUT), dtype=np.float32)
    for c in range(N_CORES):
        o0 = (c % O_WAYS) * OSH
        t0 = (c // O_WAYS) * TSH
        full[t0 : t0 + TSH, o0 : o0 + OSH] = res.results[c]["out"]
    return full.reshape(B, S, OUT), res


def kernel(x, qweight, scales, zeros, bias):
    out, _ = _run(x, qweight, scales, zeros, bias)
    return out


# revision 5
# speedup vs baseline: 1.9475x; 1.0711x over previous
"""GPTQ 4-bit quantized linear on 8 Trainium2 NeuronCores.

y[b,s,o] = sum_i x[b,s,i] * W[o,i] + bias[o]
  W[o,i] = (nib(qweight)[o,i] - zeros[o,i//128]) * scales[o,i//128]
  qweight int32 packs 2 nibbles in its low byte: i=2j low, i=2j+1 high.

Sharding: 4-way over out_features x 2-way over tokens (8 cores).
Per core: out shard [4096 tokens, 1024 outs].

Strategy (v4):
  - Weight prepacking on host: dequantize to bf16 W^T [i, o] k-tiles;
    x transposed, bf16, chunk-major so every DMA is one fat contiguous
    run per partition.
  - Device: W^T k-tiles resident in SBUF (64KB/partition), streamed on
    two queues (scalar/gpsimd alternating). 16 chunks of 256 tokens.
  - Phase A: chunks 0+1 run k-synchronized using all 8 PSUM banks, so
    per-k PE work (8 matmuls) outpaces the W k-tile arrival rate and
    the whole W load hides under compute. x quarters for both chunks
    interleave on the sync queue.
  - Phase B: chunks 2-15 tsub-outer / k-inner with W fully resident.
  - Drains: bias add on PSUM->SBUF, r=0 on vector, r=1 on gpsimd in
    parallel; stores issue per 512-column half on the scalar queue.
"""

from contextlib import ExitStack

import numpy as np
import ml_dtypes

import concourse.bass as bass
import concourse.mybir as mybir
import concourse.tile as tile
from concourse.bass_utils import run_bass_kernel_spmd

F32 = mybir.dt.float32
BF16 = mybir.dt.bfloat16

# Problem shape (hardcoded; kernel.py must be self-contained).
B, S, IN, OUT = 4, 2048, 4096, 4096
TOK = B * S
GROUP = 128
O_WAYS, T_WAYS = 4, 2
N_CORES = 8

TSH = TOK // T_WAYS      # tokens per core (4096)
OSH = OUT // O_WAYS      # out features per core (1024)
NK = IN // 128           # k slots (32)
CHUNK = 256              # tokens per chunk
N_CHUNK = TSH // CHUNK   # 16
N_TSUB = CHUNK // 128    # 2
RHS_W = 512
N_RHS = OSH // RHS_W     # 2

BF = ml_dtypes.bfloat16


def build_nc():
    nc = bass.Bass()
    xt_d = nc.declare_dram_parameter("xt", [N_CHUNK, 128, NK * CHUNK], BF16, isOutput=False)
    wt_d = nc.declare_dram_parameter("wt", [NK, 128, OSH], BF16, isOutput=False)
    bi_d = nc.declare_dram_parameter("bi", [128, OSH], BF16, isOutput=False)
    out_d = nc.declare_dram_parameter("out", [TSH, OSH], F32, isOutput=True)

    with tile.TileContext(nc) as tc, ExitStack() as ctx:
        P = 128
        pool_const = ctx.enter_context(tc.tile_pool(name="const", bufs=1))
        pool_wt = ctx.enter_context(tc.tile_pool(name="wt", bufs=1))
        pool_x = ctx.enter_context(tc.tile_pool(name="x", bufs=4))
        pool_ob = ctx.enter_context(tc.tile_pool(name="ob", bufs=4))
        psum_mm = ctx.enter_context(tc.tile_pool(name="psm", bufs=8, space="PSUM"))

        # ---- W load: bf16 W^T k-tiles, alternating scalar/gpsimd ----
        WT = []
        for k in range(NK):
            wt = pool_wt.tile([P, OSH], BF16, tag=f"wt{k}", name=f"wt{k}")
            eng = nc.scalar if k % 2 == 0 else nc.gpsimd
            eng.dma_start(out=wt[:], in_=wt_d[k, :, :])
            WT.append(wt)

        bias_t = pool_const.tile([P, OSH], BF16, tag="bias")
        nc.gpsimd.dma_start(out=bias_t[:], in_=bi_d[:, :])

        def make_ps(ch):
            return [
                [
                    psum_mm.tile([P, RHS_W], F32, tag="ps", name=f"ps{ch}_{t}_{r}")
                    for r in range(N_RHS)
                ]
                for t in range(N_TSUB)
            ]

        def drain(ch, tsub, ps):
            ob = pool_ob.tile([P, OSH], F32, tag="ob", name=f"ob{ch}_{tsub}")
            t0 = ch * CHUNK + tsub * P
            for r in range(N_RHS):
                nc.vector.tensor_add(
                    ob[:, r * RHS_W : (r + 1) * RHS_W],
                    ps[tsub][r][:],
                    bias_t[:, r * RHS_W : (r + 1) * RHS_W],
                )
                nc.scalar.dma_start(
                    out=out_d[t0 : t0 + P, r * RHS_W : (r + 1) * RHS_W],
                    in_=ob[:, r * RHS_W : (r + 1) * RHS_W],
                )

        # ---- phase A: chunks 0+1 k-synchronized on all 8 PSUM banks ----
        xcs = {}
        for ch in range(2):
            xcs[ch] = pool_x.tile([P, NK * CHUNK], BF16, tag="xc", name=f"xc{ch}")
        qstep = (NK * CHUNK) // 4
        for q in range(4):
            for ch in range(2):
                nc.sync.dma_start(
                    out=xcs[ch][:, q * qstep : (q + 1) * qstep],
                    in_=xt_d[ch, :, q * qstep : (q + 1) * qstep],
                )
        psA = {ch: make_ps(ch) for ch in range(2)}
        for k in range(NK):
            for ch in range(2):
                for tsub in range(N_TSUB):
                    lhsT = xcs[ch][:, k * CHUNK + tsub * P : k * CHUNK + (tsub + 1) * P]
                    for r in range(N_RHS):
                        nc.tensor.matmul(
                            psA[ch][tsub][r][:],
                            lhsT,
                            WT[k][:, r * RHS_W : (r + 1) * RHS_W],
                            start=(k == 0),
                            stop=(k == NK - 1),
                        )
        for ch in range(2):
            for tsub in range(N_TSUB):
                drain(ch, tsub, psA[ch])

        # ---- phase B: chunks 2-15, tsub-outer / k-inner ----
        for ch in range(2, N_CHUNK):
            xc = pool_x.tile([P, NK * CHUNK], BF16, tag="xc", name=f"xc{ch}")
            nc.sync.dma_start(out=xc[:], in_=xt_d[ch, :, :])
            ps = make_ps(ch)
            for tsub in range(N_TSUB):
                for k in range(NK):
                    lhsT = xc[:, k * CHUNK + tsub * P : k * CHUNK + (tsub + 1) * P]
                    for r in range(N_RHS):
                        nc.tensor.matmul(
                            ps[tsub][r][:],
                            lhsT,
                            WT[k][:, r * RHS_W : (r + 1) * RHS_W],
                            start=(k == 0),
                            stop=(k == NK - 1),
                        )
                drain(ch, tsub, ps)
    _legalize_waits(nc)
    return nc


_SPLIT_TYPES = (
    "InstTensorTensor",
    "InstTensorScalarPtr",
    "InstTensorScalar",
    "InstActivation",
    "InstTensorCopy",
    "InstMatmult",
    "InstDMACopy",
    "InstDrain",
)


def _legalize_waits(nc):
    """walrus allows only one on-inst sync wait for DVE/ACT elementwise
    instruction encodings; split extra waits onto same-engine Drains."""
    f = nc.m.functions[0]
    n = 0
    for blk in f.blocks:
        out_insts = []
        for inst in blk.instructions:
            si = inst.sync_info
            if (
                si is not None
                and len(si.on_wait) > 1
                and type(inst).__name__ in _SPLIT_TYPES
            ):
                waits = list(si.on_wait)
                for w in waits[:-1]:
                    d = mybir.InstDrain(name=f"waitfix{n}", ins=[], outs=[])
                    d.engine = inst.engine
                    d.sync_info = mybir.SyncInfo(on_wait=[w], on_update=[])
                    out_insts.append(d)
                    n += 1
                inst.sync_info = mybir.SyncInfo(
                    on_wait=[waits[-1]], on_update=list(si.on_update)
                )
            out_insts.append(inst)
        blk.instructions = out_insts


_NC_CACHE = {}


def _get_nc(key=()):
    if key not in _NC_CACHE:
        _NC_CACHE[key] = build_nc()
    return _NC_CACHE[key]


def make_in_maps(x, qweight, scales, zeros, bias):
    x2 = np.asarray(x).reshape(TOK, IN)
    qweight = np.asarray(qweight)
    scales = np.asarray(scales)
    zeros = np.asarray(zeros)
    bias = np.asarray(bias)

    # x: per token-shard, transpose, bf16, chunk-major [ch, p, k, t] so
    # each partition's per-chunk data is one contiguous 16KB run.
    xt_shards = []
    for t in range(T_WAYS):
        xs = x2[t * TSH : (t + 1) * TSH]                     # [TSH, IN]
        xtp = np.ascontiguousarray(xs.T).astype(BF)          # [IN, TSH]
        xtp = xtp.reshape(NK, 128, N_CHUNK, CHUNK).transpose(2, 1, 0, 3)
        xt_shards.append(np.ascontiguousarray(xtp.reshape(N_CHUNK, 128, NK * CHUNK)))

    in_maps = []
    for c in range(N_CORES):
        o0 = (c % O_WAYS) * OSH
        qw = qweight[o0 : o0 + OSH]                          # [OSH, IN//2] int32
        nib = np.empty((OSH, IN), np.float32)
        nib[:, 0::2] = (qw & 15).astype(np.float32)
        nib[:, 1::2] = ((qw >> 4) & 15).astype(np.float32)
        s = scales[o0 : o0 + OSH].astype(np.float32)         # [OSH, 32]
        z = zeros[o0 : o0 + OSH].astype(np.float32)
        w = (nib.reshape(OSH, NK, GROUP) - z[:, :, None]) * s[:, :, None]
        wt = w.reshape(OSH, IN).T.astype(BF)                 # [IN, OSH] bf16
        wt_d = np.ascontiguousarray(wt).reshape(NK, 128, OSH)

        in_maps.append(
            {
                "xt": xt_shards[c // O_WAYS],
                "wt": wt_d,
                "bi": np.ascontiguousarray(
                    np.broadcast_to(bias[o0 : o0 + OSH], (128, OSH))
                ).astype(BF),
            }
        )
    return in_maps


def _run(x, qweight, scales, zeros, bias, trace=False, **kw):
    nc = _get_nc()
    in_maps = make_in_maps(x, qweight, scales, zeros, bias)
    res = run_bass_kernel_spmd(nc, in_maps, list(range(N_CORES)), trace=trace, **kw)
    full = np.empty((TOK, OUT), dtype=np.float32)
    for c in range(N_CORES):
        o0 = (c % O_WAYS) * OSH
        t0 = (c // O_WAYS) * TSH
        full[t0 : t0 + TSH, o0 : o0 + OSH] = res.results[c]["out"]
    return full.reshape(B, S, OUT), res


def kernel(x, qweight, scales, zeros, bias):
    out, _ = _run(x, qweight, scales, zeros, bias)
    return out


# revision 7
# speedup vs baseline: 1.9587x; 1.0058x over previous
"""GPTQ 4-bit quantized linear on 8 Trainium2 NeuronCores.

y[b,s,o] = sum_i x[b,s,i] * W[o,i] + bias[o]
  W[o,i] = (nib(qweight)[o,i] - zeros[o,i//128]) * scales[o,i//128]
  qweight int32 packs 2 nibbles in its low byte: i=2j low, i=2j+1 high.

Sharding: 4-way over out_features x 2-way over tokens (8 cores).
Per core: out shard [4096 tokens, 1024 outs].

Strategy (v4):
  - Weight prepacking on host: dequantize to bf16 W^T [i, o] k-tiles;
    x transposed, bf16, chunk-major so every DMA is one fat contiguous
    run per partition.
  - Device: W^T k-tiles resident in SBUF (64KB/partition), streamed on
    two queues (scalar/gpsimd alternating). 16 chunks of 256 tokens.
  - Phase A: chunks 0+1 run k-synchronized using all 8 PSUM banks, so
    per-k PE work (8 matmuls) outpaces the W k-tile arrival rate and
    the whole W load hides under compute. x quarters for both chunks
    interleave on the sync queue.
  - Phase B: chunks 2-15 tsub-outer / k-inner with W fully resident.
  - Drains: bias add on PSUM->SBUF, r=0 on vector, r=1 on gpsimd in
    parallel; stores issue per 512-column half on the scalar queue.
"""

from contextlib import ExitStack

import numpy as np
import ml_dtypes

import concourse.bass as bass
import concourse.mybir as mybir
import concourse.tile as tile
from concourse.bass_utils import run_bass_kernel_spmd

F32 = mybir.dt.float32
BF16 = mybir.dt.bfloat16

# Problem shape (hardcoded; kernel.py must be self-contained).
B, S, IN, OUT = 4, 2048, 4096, 4096
TOK = B * S
GROUP = 128
O_WAYS, T_WAYS = 4, 2
N_CORES = 8

TSH = TOK // T_WAYS      # tokens per core (4096)
OSH = OUT // O_WAYS      # out features per core (1024)
NK = IN // 128           # k slots (32)
CHUNK = 256              # tokens per chunk
N_CHUNK = TSH // CHUNK   # 16
N_TSUB = CHUNK // 128    # 2
RHS_W = 512
N_RHS = OSH // RHS_W     # 2

BF = ml_dtypes.bfloat16


def build_nc():
    nc = bass.Bass()
    xt_d = nc.declare_dram_parameter("xt", [N_CHUNK, 128, NK * CHUNK], BF16, isOutput=False)
    wt_d = nc.declare_dram_parameter("wt", [NK, 128, OSH], BF16, isOutput=False)
    bi_d = nc.declare_dram_parameter("bi", [128, OSH], BF16, isOutput=False)
    out_d = nc.declare_dram_parameter("out", [TSH, OSH], F32, isOutput=True)

    with tile.TileContext(nc) as tc, ExitStack() as ctx:
        P = 128
        pool_const = ctx.enter_context(tc.tile_pool(name="const", bufs=1))
        pool_wt = ctx.enter_context(tc.tile_pool(name="wt", bufs=1))
        pool_x = ctx.enter_context(tc.tile_pool(name="x", bufs=4))
        pool_ob = ctx.enter_context(tc.tile_pool(name="ob", bufs=4))
        psum_mm = ctx.enter_context(tc.tile_pool(name="psm", bufs=8, space="PSUM"))

        # ---- W load: bf16 W^T k-tiles, alternating scalar/gpsimd ----
        WT = []
        for k in range(NK):
            wt = pool_wt.tile([P, OSH], BF16, tag=f"wt{k}", name=f"wt{k}")
            eng = nc.scalar if k % 2 == 0 else nc.gpsimd
            eng.dma_start(out=wt[:], in_=wt_d[k, :, :])
            WT.append(wt)

        bias_t = pool_const.tile([P, OSH], BF16, tag="bias")
        nc.gpsimd.dma_start(out=bias_t[:], in_=bi_d[:, :])

        def make_ps(ch):
            return [
                [
                    psum_mm.tile([P, RHS_W], F32, tag="ps", name=f"ps{ch}_{t}_{r}")
                    for r in range(N_RHS)
                ]
                for t in range(N_TSUB)
            ]

        def drain(ch, tsub, ps):
            ob = pool_ob.tile([P, OSH], F32, tag="ob", name=f"ob{ch}_{tsub}")
            t0 = ch * CHUNK + tsub * P
            for r in range(N_RHS):
                nc.vector.tensor_add(
                    ob[:, r * RHS_W : (r + 1) * RHS_W],
                    ps[tsub][r][:],
                    bias_t[:, r * RHS_W : (r + 1) * RHS_W],
                )
                eng = nc.scalar if r == 0 else nc.sync
                eng.dma_start(
                    out=out_d[t0 : t0 + P, r * RHS_W : (r + 1) * RHS_W],
                    in_=ob[:, r * RHS_W : (r + 1) * RHS_W],
                )

        # ---- phase A: chunks 0+1 k-synchronized on all 8 PSUM banks ----
        xcs = {}
        for ch in range(2):
            xcs[ch] = pool_x.tile([P, NK * CHUNK], BF16, tag="xc", name=f"xc{ch}")
        qstep = (NK * CHUNK) // 8
        for q in range(8):
            for ch in range(2):
                nc.sync.dma_start(
                    out=xcs[ch][:, q * qstep : (q + 1) * qstep],
                    in_=xt_d[ch, :, q * qstep : (q + 1) * qstep],
                )
        psA = {ch: make_ps(ch) for ch in range(2)}
        for k in range(NK):
            for ch in range(2):
                for tsub in range(N_TSUB):
                    lhsT = xcs[ch][:, k * CHUNK + tsub * P : k * CHUNK + (tsub + 1) * P]
                    for r in range(N_RHS):
                        nc.tensor.matmul(
                            psA[ch][tsub][r][:],
                            lhsT,
                            WT[k][:, r * RHS_W : (r + 1) * RHS_W],
                            start=(k == 0),
                            stop=(k == NK - 1),
                        )
        for ch in range(2):
            for tsub in range(N_TSUB):
                drain(ch, tsub, psA[ch])

        # ---- phase B: chunks 2-15, tsub-outer / k-inner ----
        for ch in range(2, N_CHUNK):
            xc = pool_x.tile([P, NK * CHUNK], BF16, tag="xc", name=f"xc{ch}")
            nc.sync.dma_start(out=xc[:], in_=xt_d[ch, :, :])
            ps = make_ps(ch)
            for tsub in range(N_TSUB):
                for k in range(NK):
                    lhsT = xc[:, k * CHUNK + tsub * P : k * CHUNK + (tsub + 1) * P]
                    for r in range(N_RHS):
                        nc.tensor.matmul(
                            ps[tsub][r][:],
                            lhsT,
                            WT[k][:, r * RHS_W : (r + 1) * RHS_W],
                            start=(k == 0),
                            stop=(k == NK - 1),
                        )
                drain(ch, tsub, ps)
    _legalize_waits(nc)
    return nc


_SPLIT_TYPES = (
    "InstTensorTensor",
    "InstTensorScalarPtr",
    "InstTensorScalar",
    "InstActivation",
    "InstTensorCopy",
    "InstMatmult",
    "InstDMACopy",
    "InstDrain",
)


def _legalize_waits(nc):
    """walrus allows only one on-inst sync wait for DVE/ACT elementwise
    instruction encodings; split extra waits onto same-engine Drains."""
    f = nc.m.functions[0]
    n = 0
    for blk in f.blocks:
        out_insts = []
        for inst in blk.instructions:
            si = inst.sync_info
            if (
                si is not None
                and len(si.on_wait) > 1
                and type(inst).__name__ in _SPLIT_TYPES
            ):
                waits = list(si.on_wait)
                for w in waits[:-1]:
                    d = mybir.InstDrain(name=f"waitfix{n}", ins=[], outs=[])
                    d.engine = inst.engine
                    d.sync_info = mybir.SyncInfo(on_wait=[w], on_update=[])
                    out_insts.append(d)
                    n += 1
                inst.sync_info = mybir.SyncInfo(
                    on_wait=[waits[-1]], on_update=list(si.on_update)
                )
            out_insts.append(inst)
        blk.instructions = out_insts


_NC_CACHE = {}


def _get_nc(key=()):
    if key not in _NC_CACHE:
        _NC_CACHE[key] = build_nc()
    return _NC_CACHE[key]


def make_in_maps(x, qweight, scales, zeros, bias):
    x2 = np.asarray(x).reshape(TOK, IN)
    qweight = np.asarray(qweight)
    scales = np.asarray(scales)
    zeros = np.asarray(zeros)
    bias = np.asarray(bias)

    # x: per token-shard, transpose, bf16, chunk-major [ch, p, k, t] so
    # each partition's per-chunk data is one contiguous 16KB run.
    xt_shards = []
    for t in range(T_WAYS):
        xs = x2[t * TSH : (t + 1) * TSH]                     # [TSH, IN]
        xtp = np.ascontiguousarray(xs.T).astype(BF)          # [IN, TSH]
        xtp = xtp.reshape(NK, 128, N_CHUNK, CHUNK).transpose(2, 1, 0, 3)
        xt_shards.append(np.ascontiguousarray(xtp.reshape(N_CHUNK, 128, NK * CHUNK)))

    in_maps = []
    for c in range(N_CORES):
        o0 = (c % O_WAYS) * OSH
        qw = qweight[o0 : o0 + OSH]                          # [OSH, IN//2] int32
        nib = np.empty((OSH, IN), np.float32)
        nib[:, 0::2] = (qw & 15).astype(np.float32)
        nib[:, 1::2] = ((qw >> 4) & 15).astype(np.float32)
        s = scales[o0 : o0 + OSH].astype(np.float32)         # [OSH, 32]
        z = zeros[o0 : o0 + OSH].astype(np.float32)
        w = (nib.reshape(OSH, NK, GROUP) - z[:, :, None]) * s[:, :, None]
        wt = w.reshape(OSH, IN).T.astype(BF)                 # [IN, OSH] bf16
        wt_d = np.ascontiguousarray(wt).reshape(NK, 128, OSH)

        in_maps.append(
            {
                "xt": xt_shards[c // O_WAYS],
                "wt": wt_d,
                "bi": np.ascontiguousarray(
                    np.broadcast_to(bias[o0 : o0 + OSH], (128, OSH))
                ).astype(BF),
            }
        )
    return in_maps


def _run(x, qweight, scales, zeros, bias, trace=False, **kw):
    nc = _get_nc()
    in_maps = make_in_maps(x, qweight, scales, zeros, bias)
    res = run_bass_kernel_spmd(nc, in_maps, list(range(N_CORES)), trace=trace, **kw)
    full = np.empty((TOK, OUT), dtype=np.float32)
    for c in range(N_CORES):
        o0 = (c % O_WAYS) * OSH
        t0 = (c // O_WAYS) * TSH
        full[t0 : t0 + TSH, o0 : o0 + OSH] = res.results[c]["out"]
    return full.reshape(B, S, OUT), res


def kernel(x, qweight, scales, zeros, bias):
    out, _ = _run(x, qweight, scales, zeros, bias)
    return out
